# revision 9
# baseline (speedup 1.0000x reference)
"""AlibiCausalSelfAttention on 8 Trainium2 NeuronCores.

Sharding: data-parallel over batch (B=2) x head-parallel over head groups
(16 heads -> 4 groups of 4). Core c handles batch c//4, heads [4*(c%4), 4*(c%4)+4).
Each core computes a partial projection output [T, C] (W_proj row-sharded);
the host sums the 4 partials per batch and adds b_proj.

Per-core kernel layout (T=2048, C=1024, D=64, 4 local heads):
  phase 1: qkT [512, T] = (Wqk^T x^T) via matmul(lhsT=Wqk chunk, rhs=xT chunk),
           v [T, 256] via matmul(lhsT=xT chunk, rhs=Wv chunk). Biases are added
           with K=1 matmuls against a ones row. q columns pre-scaled by 1/sqrt(D)
           on host. q/k stored per head as [128, T] tiles with 2 extra contraction
           rows encoding ALiBi: St = k_aug^T q_aug = q.k/8 + slope*(j-i) exactly
           (slopes are powers of two).
  phase 2: per (head, i-tile of 512): St[j,i] chunks [128, W], exp on ACT,
           causal diag squares masked by multiplying an upper-triangular 0/1
           matrix, PV accumulated as yT[d, i] with an appended ones column in v
           producing the softmax denominator in psum row 64. Normalization:
           reciprocal of the denom row, broadcast across partitions with a K=1
           matmul, multiplied into yT.
  phase 3: out[t, e] = yT^T Wp via matmul(lhsT=yT chunk, rhs=Wp chunk).

Matmul operands are bitcast to float32r (full-rate fp32 matmul mode).
"""

import sys

if "/opt/trn_rl_repo" not in sys.path:
    sys.path.insert(0, "/opt/trn_rl_repo")

import numpy as np

T = 2048
C = 1024
H = 16
D = 64
HL = 4          # heads per core
HD = HL * D     # 256 local head dims
IW = 512        # i-tile width
NEG = None      # causal handled structurally, no big-negative constant needed

_CACHE = {}


def _build_nc(mm_dt_name="float32r"):
    import concourse.mybir as mybir
    import concourse.tile as tile
    from concourse import bacc
    from contextlib import ExitStack

    f32 = mybir.dt.float32
    fr = getattr(mybir.dt, mm_dt_name)
    mm_dt = fr
    Exp = mybir.ActivationFunctionType.Exp

    nc = bacc.Bacc("TRN2", target_bir_lowering=False, debug=False, num_devices=8)

    xT = nc.dram_tensor("xT", [C, T], fr, kind="ExternalInput").ap()
    wqk = nc.dram_tensor("wqk", [C, 2 * HD], fr, kind="ExternalInput").ap()
    bqk = nc.dram_tensor("bqk", [1, 2 * HD], fr, kind="ExternalInput").ap()
    wv = nc.dram_tensor("wv", [C, HD], fr, kind="ExternalInput").ap()
    bv = nc.dram_tensor("bv", [1, HD], fr, kind="ExternalInput").ap()
    wp = nc.dram_tensor("wp", [HD, C], fr, kind="ExternalInput").ap()
    aq = nc.dram_tensor("aq", [2 * HL, T], fr, kind="ExternalInput").ap()
    ak = nc.dram_tensor("ak", [2, T], fr, kind="ExternalInput").ap()
    up01 = nc.dram_tensor("up01", [128, 128], fr, kind="ExternalInput").ap()
    onec = nc.dram_tensor("onec", [128, 1], fr, kind="ExternalInput").ap()
    out = nc.dram_tensor("out", [T, C], f32, kind="ExternalOutput").ap()

    def r(ap):
        return ap

    with tile.TileContext(nc) as tc, ExitStack() as ctx:
        pers = ctx.enter_context(tc.tile_pool(name="pers", bufs=1))

        qaug = [pers.tile([128, T], fr, tag=f"qaug{h}", name=f"qaug{h}") for h in range(HL)]
        kaug = [pers.tile([128, T], fr, tag=f"kaug{h}", name=f"kaug{h}") for h in range(HL)]
        # v tiles per (head, t-chunk): [128, 65], col 64 = ones (denominator trick)
        vaug = [[pers.tile([128, 65], fr, tag=f"vaug{h}_{t}", name=f"vaug{h}_{t}") for t in range(T // 128)]
                for h in range(HL)]
        yT = [pers.tile([128, T], fr, tag=f"yT{i}", name=f"yT{i}") for i in range(HL // 2)]
        wp_sb = [pers.tile([128, C], fr, tag=f"wp{i}", name=f"wp{i}") for i in range(2)]
        up = pers.tile([128, 128], fr, tag="up")
        onescol = pers.tile([128, 1], fr, tag="onescol")
        onesrow = pers.tile([1, IW], fr, tag="onesrow")
        nc.sync.dma_start(up[:], up01[:])
        nc.sync.dma_start(onescol[:], onec[:])
        nc.sync.dma_start(onesrow[:], ak[0:1, 0:IW])
        for i in range(2):
            nc.sync.dma_start(wp_sb[i][:], wp[i * 128:(i + 1) * 128, :])
        for h in range(HL):
            # zero the padding rows once; alibi rows come from host arrays
            nc.sync.dma_start(qaug[h][64:66, :], aq[2 * h:2 * h + 2, :])
            nc.sync.dma_start(kaug[h][64:66, :], ak[:, :])
            for t in range(T // 128):
                nc.vector.tensor_copy(vaug[h][t][:, 64:65], onescol[:])

        # ---------------- phase 1: projections ----------------
        with tc.tile_pool(name="ph1", bufs=1) as ph1, \
             tc.tile_pool(name="ps1", bufs=4, space="PSUM") as ps1:
            xs = []
            for k in range(8):
                t_ = ph1.tile([128, T], fr, tag=f"x{k}", name=f"x{k}")
                nc.sync.dma_start(t_[:], xT[k * 128:(k + 1) * 128, :])
                xs.append(t_)
            wqks = []
            for k in range(8):
                t_ = ph1.tile([128, 2 * HD], fr, tag=f"wqk{k}", name=f"wqk{k}")
                nc.sync.dma_start(t_[:], wqk[k * 128:(k + 1) * 128, :])
                wqks.append(t_)
            wvs = []
            for k in range(8):
                t_ = ph1.tile([128, HD], fr, tag=f"wv{k}", name=f"wv{k}")
                nc.sync.dma_start(t_[:], wv[k * 128:(k + 1) * 128, :])
                wvs.append(t_)
            bqk_sb = ph1.tile([1, 2 * HD], fr, tag="bqk")
            nc.sync.dma_start(bqk_sb[:], bqk[:])
            bv_sb = ph1.tile([1, HD], fr, tag="bv")
            nc.sync.dma_start(bv_sb[:], bv[:])

            # qkT [512, T]: col-chunk cc (0,1 = q heads; 2,3 = k heads)
            for cc in range(4):
                for tt in range(4):
                    ps = ps1.tile([128, IW], f32, tag="qk", name="qkps")
                    for k in range(8):
                        nc.tensor.matmul(
                            ps[:],
                            r(wqks[k][:, cc * 128:(cc + 1) * 128]),
                            r(xs[k][:, tt * IW:(tt + 1) * IW]),
                            start=(k == 0), stop=False)
                    nc.tensor.matmul(
                        ps[:], r(bqk_sb[:, cc * 128:(cc + 1) * 128]), r(onesrow[:, 0:IW]),
                        start=False, stop=True)
                    for half in range(2):
                        h = (cc % 2) * 2 + half
                        dst = qaug[h] if cc < 2 else kaug[h]
                        nc.vector.tensor_copy(
                            dst[0:64, tt * IW:(tt + 1) * IW],
                            ps[half * 64:(half + 1) * 64, :])

            # v [T, 256] natural layout
            for t16 in range(T // 128):
                ps = ps1.tile([128, HD], f32, tag="v", name="vps")
                for k in range(8):
                    nc.tensor.matmul(
                        ps[:],
                        r(xs[k][:, t16 * 128:(t16 + 1) * 128]),
                        r(wvs[k][:]),
                        start=(k == 0), stop=False)
                nc.tensor.matmul(
                    ps[:], r(onesrow[:, 0:128]), r(bv_sb[:]), start=False, stop=True)
                for h in range(HL):
                    nc.vector.tensor_copy(
                        vaug[h][t16][:, 0:64], ps[:, h * 64:(h + 1) * 64])

        # ---------------- phase 2: attention ----------------
        with tc.tile_pool(name="ph2", bufs=3) as ph2, \
             tc.tile_pool(name="ps2", bufs=2, space="PSUM") as ps2, \
             tc.tile_pool(name="py", bufs=2, space="PSUM") as py:
            for h in range(HL):
                for it in range(T // IW):
                    i0 = it * IW
                    njc = i0 // 128 + IW // 128
                    yacc = py.tile([65, IW], f32, tag="yacc", name="yacc")
                    for jc in range(njc):
                        j0 = jc * 128
                        c0 = max(0, j0 - i0)
                        W = IW - c0
                        st = ps2.tile([128, IW], f32, tag="st", name="st")[:, 0:W]
                        nc.tensor.matmul(
                            st[:],
                            r(kaug[h][0:66, j0:j0 + 128]),
                            r(qaug[h][0:66, i0 + c0:i0 + IW]),
                            start=True, stop=True)
                        p = ph2.tile([128, IW], fr, tag="p", name="p")[:, 0:W]
                        nc.scalar.activation(p[:], st[:], Exp)
                        if j0 >= i0:
                            nc.vector.tensor_mul(p[:, 0:128], p[:, 0:128], up[:])
                        nc.tensor.matmul(
                            yacc[:, c0:IW], r(vaug[h][jc][:]), r(p[:]),
                            start=(jc == 0), stop=(jc == njc - 1))
                    rec = ph2.tile([1, IW], f32, tag="rec", name="rec")
                    nc.vector.reciprocal(rec[:], yacc[64:65, :])
                    recr = ph2.tile([1, IW], fr, tag="recr", name="recr")
                    nc.vector.tensor_copy(recr[:], rec[:])
                    r64 = ps2.tile([64, IW], f32, tag="r64", name="r64")
                    nc.tensor.matmul(r64[:], r(onesrow[:, 0:64]), r(recr[:]),
                                     start=True, stop=True)
                    r64s = ph2.tile([64, IW], f32, tag="r64s", name="r64s")
                    nc.vector.tensor_copy(r64s[:], r64[:])
                    nc.vector.tensor_mul(
                        yT[h // 2][(h % 2) * 64:(h % 2) * 64 + 64, i0:i0 + IW],
                        yacc[0:64, :], r64s[:])

        # ---------------- phase 3: output projection ----------------
        with tc.tile_pool(name="ph3", bufs=3) as ph3, \
             tc.tile_pool(name="ps3", bufs=2, space="PSUM") as ps3:
            for t16 in range(T // 128):
                for e2 in range(2):
                    ps = ps3.tile([128, 512], f32, tag="o", name="ops")
                    for kk in range(2):
                        nc.tensor.matmul(
                            ps[:],
                            r(yT[kk][:, t16 * 128:(t16 + 1) * 128]),
                            r(wp_sb[kk][:, e2 * 512:(e2 + 1) * 512]),
                            start=(kk == 0), stop=(kk == 1))
                    ot = ph3.tile([128, 512], f32, tag="ot", name="ot")
                    nc.vector.tensor_copy(ot[:], ps[:])
                    nc.sync.dma_start(
                        out[t16 * 128:(t16 + 1) * 128, e2 * 512:(e2 + 1) * 512],
                        ot[:])

    nc.compile()
    return nc


def _get_nc():
    if "nc" not in _CACHE:
        _CACHE["nc"] = _build_nc()
    return _CACHE["nc"]


def _shard_inputs(x, W_attn, b_attn, W_proj, b_proj):
    slopes = (1.0 / np.power(2.0, np.arange(1, H + 1))).astype(np.float32)
    iota = np.arange(T, dtype=np.float32)
    ak = np.stack([np.ones(T, np.float32), iota])                  # [2, T]
    up01 = np.triu(np.ones((128, 128), np.float32))                # keep j<=i
    xTs = [np.ascontiguousarray(x[b].T) for b in range(x.shape[0])]

    in_maps = []
    for core in range(8):
        b, g = core // 4, core % 4
        cs = slice(g * HD, (g + 1) * HD)
        q_cols = W_attn[:, 0:C][:, cs] * 0.125
        k_cols = W_attn[:, C:2 * C][:, cs]
        v_cols = np.ascontiguousarray(W_attn[:, 2 * C:3 * C][:, cs])
        wqk_l = np.ascontiguousarray(np.concatenate([q_cols, k_cols], axis=1))
        bqk_l = np.concatenate(
            [b_attn[0:C][cs] * 0.125, b_attn[C:2 * C][cs]])[None, :]
        bv_l = b_attn[2 * C:3 * C][cs][None, :]
        wp_l = np.ascontiguousarray(W_proj[g * HD:(g + 1) * HD, :])
        aq = np.zeros((2 * HL, T), np.float32)
        for hh in range(HL):
            s = slopes[g * HL + hh]
            aq[2 * hh, :] = -s * iota
            aq[2 * hh + 1, :] = s
        in_maps.append({
            "xT": xTs[b], "wqk": wqk_l,
            "bqk": np.ascontiguousarray(bqk_l, dtype=np.float32),
            "wv": v_cols, "bv": np.ascontiguousarray(bv_l, dtype=np.float32),
            "wp": wp_l, "aq": aq, "ak": ak, "up01": up01,
            "onec": np.ones((128, 1), np.float32),
        })
    return in_maps


def kernel(x, W_attn, b_attn, W_proj, b_proj, _trace=False, _tmpdir=None):
    from concourse.bass_utils import run_bass_kernel_spmd

    x = np.asarray(x, dtype=np.float32)
    W_attn = np.asarray(W_attn, dtype=np.float32)
    b_attn = np.asarray(b_attn, dtype=np.float32)
    W_proj = np.asarray(W_proj, dtype=np.float32)
    b_proj = np.asarray(b_proj, dtype=np.float32)

    nc = _get_nc()
    in_maps = _shard_inputs(x, W_attn, b_attn, W_proj, b_proj)
    res = run_bass_kernel_spmd(
        nc, in_maps, core_ids=list(range(8)), trace=_trace, tmpdir=_tmpdir)
    parts = [res.results[i]["out"] for i in range(8)]
    out = np.empty((x.shape[0], T, C), np.float32)
    for b in range(x.shape[0]):
        out[b] = parts[4 * b] + parts[4 * b + 1] + parts[4 * b + 2] + parts[4 * b + 3]
        out[b] += b_proj
    if _trace:
        kernel.last_exec_time_ns = res.exec_time_ns
    return out


# revision 11
# speedup vs baseline: 1.1939x; 1.1939x over previous
"""AlibiCausalSelfAttention on 8 Trainium2 NeuronCores.

Sharding: data-parallel over batch (B=2) x head-parallel over head groups
(16 heads -> 4 groups of 4). Core c handles batch c//4, heads [4*(c%4), 4*(c%4)+4).
Each core computes a partial projection output [T, C] (W_proj row-sharded);
the host sums the 4 partials per batch and adds b_proj.

Per-core kernel layout (T=2048, C=1024, D=64, 4 local heads):
  phase 1: qkT [512, T] = (Wqk^T x^T) via matmul(lhsT=Wqk chunk, rhs=xT chunk),
           v [T, 256] via matmul(lhsT=xT chunk, rhs=Wv chunk). Biases are added
           with K=1 matmuls against a ones row. q columns pre-scaled by 1/sqrt(D)
           on host. q/k stored per head as [128, T] tiles with 2 extra contraction
           rows encoding ALiBi: St = k_aug^T q_aug = q.k/8 + slope*(j-i) exactly
           (slopes are powers of two).
  phase 2: per (head, i-tile of 512): St[j,i] chunks [128, W], exp on ACT,
           causal diag squares masked by multiplying an upper-triangular 0/1
           matrix, PV accumulated as yT[d, i] with an appended ones column in v
           producing the softmax denominator in psum row 64. Normalization:
           reciprocal of the denom row, broadcast across partitions with a K=1
           matmul, multiplied into yT.
  phase 3: out[t, e] = yT^T Wp via matmul(lhsT=yT chunk, rhs=Wp chunk).

Matmul operands are bitcast to float32r (full-rate fp32 matmul mode).
"""

import sys

if "/opt/trn_rl_repo" not in sys.path:
    sys.path.insert(0, "/opt/trn_rl_repo")

import numpy as np

T = 2048
C = 1024
H = 16
D = 64
HL = 4          # heads per core
HD = HL * D     # 256 local head dims
IW = 512        # i-tile width
NEG = None      # causal handled structurally, no big-negative constant needed

_CACHE = {}


def _build_nc(mm_dt_name="float32r"):
    import concourse.mybir as mybir
    import concourse.tile as tile
    from concourse import bacc
    from contextlib import ExitStack

    f32 = mybir.dt.float32
    fr = mybir.dt.float16
    mm_dt = fr
    Exp = mybir.ActivationFunctionType.Exp

    nc = bacc.Bacc("TRN2", target_bir_lowering=False, debug=False, num_devices=8)

    xT = nc.dram_tensor("xT", [C, T], fr, kind="ExternalInput").ap()
    wqk = nc.dram_tensor("wqk", [C, 2 * HD], fr, kind="ExternalInput").ap()
    bqk = nc.dram_tensor("bqk", [1, 2 * HD], fr, kind="ExternalInput").ap()
    wv = nc.dram_tensor("wv", [C, HD], fr, kind="ExternalInput").ap()
    bv = nc.dram_tensor("bv", [1, HD], fr, kind="ExternalInput").ap()
    wp = nc.dram_tensor("wp", [HD, C], fr, kind="ExternalInput").ap()
    aq = nc.dram_tensor("aq", [2 * HL, T], fr, kind="ExternalInput").ap()
    ak = nc.dram_tensor("ak", [2, T], fr, kind="ExternalInput").ap()
    upx_d = nc.dram_tensor("upx", [128, IW], f32, kind="ExternalInput").ap()
    onec = nc.dram_tensor("onec", [128, 1], fr, kind="ExternalInput").ap()
    onesr_d = nc.dram_tensor("onesr", [1, IW], fr, kind="ExternalInput").ap()
    out = nc.dram_tensor("out", [T, C], f32, kind="ExternalOutput").ap()

    def r(ap):
        return ap

    with tile.TileContext(nc) as tc, ExitStack() as ctx:
        pers = ctx.enter_context(tc.tile_pool(name="pers", bufs=1))

        qaug = [pers.tile([128, T], fr, tag=f"qaug{h}", name=f"qaug{h}") for h in range(HL)]
        kaug = [pers.tile([128, T], fr, tag=f"kaug{h}", name=f"kaug{h}") for h in range(HL)]
        # v tiles per (head, t-chunk): [128, 65], col 64 = ones (denominator trick)
        vaug = [[pers.tile([128, 65], fr, tag=f"vaug{h}_{t}", name=f"vaug{h}_{t}") for t in range(T // 128)]
                for h in range(HL)]
        yT = [pers.tile([128, T], fr, tag=f"yT{i}", name=f"yT{i}") for i in range(HL // 2)]
        wp_sb = [pers.tile([128, C], fr, tag=f"wp{i}", name=f"wp{i}") for i in range(2)]
        upx = pers.tile([128, IW], f32, tag="upx")
        onescol = pers.tile([128, 1], fr, tag="onescol")
        onesrow = pers.tile([1, IW], fr, tag="onesrow")
        nc.sync.dma_start(upx[:], upx_d[:])
        nc.sync.dma_start(onescol[:], onec[:])
        nc.sync.dma_start(onesrow[:], onesr_d[:])
        for i in range(2):
            nc.sync.dma_start(wp_sb[i][:], wp[i * 128:(i + 1) * 128, :])
        for h in range(HL):
            # zero the padding rows once; alibi rows come from host arrays
            nc.sync.dma_start(qaug[h][64:66, :], aq[2 * h:2 * h + 2, :])
            nc.sync.dma_start(kaug[h][64:66, :], ak[:, :])
            for t in range(T // 128):
                nc.vector.tensor_copy(vaug[h][t][:, 64:65], onescol[:])

        # ---------------- phase 1: projections ----------------
        with tc.tile_pool(name="ph1", bufs=1) as ph1, \
             tc.tile_pool(name="ps1", bufs=4, space="PSUM") as ps1:
            xs = []
            for k in range(8):
                t_ = ph1.tile([128, T], fr, tag=f"x{k}", name=f"x{k}")
                nc.sync.dma_start(t_[:], xT[k * 128:(k + 1) * 128, :])
                xs.append(t_)
            wqks = []
            for k in range(8):
                t_ = ph1.tile([128, 2 * HD], fr, tag=f"wqk{k}", name=f"wqk{k}")
                nc.sync.dma_start(t_[:], wqk[k * 128:(k + 1) * 128, :])
                wqks.append(t_)
            wvs = []
            for k in range(8):
                t_ = ph1.tile([128, HD], fr, tag=f"wv{k}", name=f"wv{k}")
                nc.sync.dma_start(t_[:], wv[k * 128:(k + 1) * 128, :])
                wvs.append(t_)
            bqk_sb = ph1.tile([1, 2 * HD], fr, tag="bqk")
            nc.sync.dma_start(bqk_sb[:], bqk[:])
            bv_sb = ph1.tile([1, HD], fr, tag="bv")
            nc.sync.dma_start(bv_sb[:], bv[:])

            # qkT [512, T]: col-chunk cc (0,1 = q heads; 2,3 = k heads)
            for cc in range(4):
                for tt in range(4):
                    ps = ps1.tile([128, IW], f32, tag="qk", name="qkps")
                    for k in range(8):
                        nc.tensor.matmul(
                            ps[:],
                            r(wqks[k][:, cc * 128:(cc + 1) * 128]),
                            r(xs[k][:, tt * IW:(tt + 1) * IW]),
                            start=(k == 0), stop=False)
                    nc.tensor.matmul(
                        ps[:], r(bqk_sb[:, cc * 128:(cc + 1) * 128]), r(onesrow[:, 0:IW]),
                        start=False, stop=True)
                    for half in range(2):
                        h = (cc % 2) * 2 + half
                        dst = qaug[h] if cc < 2 else kaug[h]
                        nc.vector.tensor_copy(
                            dst[0:64, tt * IW:(tt + 1) * IW],
                            ps[half * 64:(half + 1) * 64, :])

            # v [T, 256] natural layout
            for t16 in range(T // 128):
                ps = ps1.tile([128, HD], f32, tag="v", name="vps")
                for k in range(8):
                    nc.tensor.matmul(
                        ps[:],
                        r(xs[k][:, t16 * 128:(t16 + 1) * 128]),
                        r(wvs[k][:]),
                        start=(k == 0), stop=False)
                nc.tensor.matmul(
                    ps[:], r(onesrow[:, 0:128]), r(bv_sb[:]), start=False, stop=True)
                for h in range(HL):
                    nc.vector.tensor_copy(
                        vaug[h][t16][:, 0:64], ps[:, h * 64:(h + 1) * 64])

        # ---------------- phase 2: attention ----------------
        with tc.tile_pool(name="ph2", bufs=3) as ph2, \
             tc.tile_pool(name="ps2", bufs=2, space="PSUM") as ps2, \
             tc.tile_pool(name="py", bufs=2, space="PSUM") as py:
            for h in range(HL):
                for it in range(T // IW):
                    i0 = it * IW
                    njc = i0 // 128 + IW // 128
                    yacc = py.tile([65, IW], f32, tag="yacc", name="yacc")
                    for jc in range(njc):
                        j0 = jc * 128
                        c0 = max(0, j0 - i0)
                        W = IW - c0
                        st = ps2.tile([128, IW], f32, tag="st", name="st")[:, 0:W]
                        nc.tensor.matmul(
                            st[:],
                            r(kaug[h][0:66, j0:j0 + 128]),
                            r(qaug[h][0:66, i0 + c0:i0 + IW]),
                            start=True, stop=True)
                        p = ph2.tile([128, IW], fr, tag="p", name="p")[:, 0:W]
                        if j0 >= i0:
                            p32 = ph2.tile([128, IW], f32, tag="p32", name="p32")[:, 0:W]
                            nc.scalar.activation(p32[:], st[:], Exp)
                            nc.vector.tensor_mul(p[:], p32[:], upx[:, 0:W])
                        else:
                            nc.scalar.activation(p[:], st[:], Exp)
                        nc.tensor.matmul(
                            yacc[:, c0:IW], r(vaug[h][jc][:]), r(p[:]),
                            start=(jc == 0), stop=(jc == njc - 1))
                    dcp = ph2.tile([1, IW], f32, tag="dcp", name="dcp")
                    nc.vector.tensor_copy(dcp[:], yacc[64:65, :])
                    rec = ph2.tile([1, IW], f32, tag="rec", name="rec")
                    scr = ph2.tile([1, IW], f32, tag="scr", name="scr")
                    nc.vector.reciprocal_approx_accurate(rec[:], dcp[:], scr[:])
                    recr = ph2.tile([1, IW], fr, tag="recr", name="recr")
                    nc.vector.tensor_copy(recr[:], rec[:])
                    r64 = ps2.tile([64, IW], f32, tag="r64", name="r64")
                    nc.tensor.matmul(r64[:], r(onesrow[:, 0:64]), r(recr[:]),
                                     start=True, stop=True)
                    r64s = ph2.tile([64, IW], f32, tag="r64s", name="r64s")
                    nc.vector.tensor_copy(r64s[:], r64[:])
                    nc.vector.tensor_mul(
                        yT[h // 2][(h % 2) * 64:(h % 2) * 64 + 64, i0:i0 + IW],
                        yacc[0:64, :], r64s[:])

        # ---------------- phase 3: output projection ----------------
        with tc.tile_pool(name="ph3", bufs=3) as ph3, \
             tc.tile_pool(name="ps3", bufs=2, space="PSUM") as ps3:
            for t16 in range(T // 128):
                for e2 in range(2):
                    ps = ps3.tile([128, 512], f32, tag="o", name="ops")
                    for kk in range(2):
                        nc.tensor.matmul(
                            ps[:],
                            r(yT[kk][:, t16 * 128:(t16 + 1) * 128]),
                            r(wp_sb[kk][:, e2 * 512:(e2 + 1) * 512]),
                            start=(kk == 0), stop=(kk == 1))
                    ot = ph3.tile([128, 512], f32, tag="ot", name="ot")
                    nc.vector.tensor_copy(ot[:], ps[:])
                    nc.sync.dma_start(
                        out[t16 * 128:(t16 + 1) * 128, e2 * 512:(e2 + 1) * 512],
                        ot[:])

    nc.compile()
    return nc


def _get_nc():
    if "nc" not in _CACHE:
        _CACHE["nc"] = _build_nc()
    return _CACHE["nc"]


def _shard_inputs(x, W_attn, b_attn, W_proj, b_proj):
    f16 = np.float16
    slopes = (1.0 / np.power(2.0, np.arange(1, H + 1))).astype(np.float32)
    iota = np.arange(T, dtype=np.float32)
    ak = np.stack([np.ones(T, np.float32), iota]).astype(f16)      # [2, T]
    # multiplicative mask for diagonal chunks: cols 0:128 upper-tri keep, rest ones
    pp, ff = np.meshgrid(np.arange(128), np.arange(128), indexing="ij")
    upx = np.ones((128, IW), np.float32)
    upx[:, 0:128] = np.where(pp <= ff, 1.0, 0.0)
    xTs = [np.ascontiguousarray(x[b].T).astype(f16) for b in range(x.shape[0])]

    in_maps = []
    for core in range(8):
        b, g = core // 4, core % 4
        cs = slice(g * HD, (g + 1) * HD)
        q_cols = W_attn[:, 0:C][:, cs] * 0.125
        k_cols = W_attn[:, C:2 * C][:, cs]
        v_cols = np.ascontiguousarray(W_attn[:, 2 * C:3 * C][:, cs])
        wqk_l = np.ascontiguousarray(np.concatenate([q_cols, k_cols], axis=1))
        bqk_l = np.concatenate(
            [b_attn[0:C][cs] * 0.125, b_attn[C:2 * C][cs]])[None, :]
        bv_l = b_attn[2 * C:3 * C][cs][None, :]
        wp_l = np.ascontiguousarray(W_proj[g * HD:(g + 1) * HD, :])
        aq = np.zeros((2 * HL, T), np.float32)
        for hh in range(HL):
            s = slopes[g * HL + hh]
            aq[2 * hh, :] = -s * iota
            aq[2 * hh + 1, :] = s
        in_maps.append({
            "xT": xTs[b], "wqk": wqk_l.astype(f16),
            "bqk": np.ascontiguousarray(bqk_l).astype(f16),
            "wv": v_cols.astype(f16), "bv": np.ascontiguousarray(bv_l).astype(f16),
            "wp": wp_l.astype(f16), "aq": aq.astype(f16), "ak": ak,
            "upx": upx, "onec": np.ones((128, 1), f16),
            "onesr": np.ones((1, IW), f16),
        })
    return in_maps


def kernel(x, W_attn, b_attn, W_proj, b_proj, _trace=False, _tmpdir=None):
    from concourse.bass_utils import run_bass_kernel_spmd

    x = np.asarray(x, dtype=np.float32)
    W_attn = np.asarray(W_attn, dtype=np.float32)
    b_attn = np.asarray(b_attn, dtype=np.float32)
    W_proj = np.asarray(W_proj, dtype=np.float32)
    b_proj = np.asarray(b_proj, dtype=np.float32)

    nc = _get_nc()
    in_maps = _shard_inputs(x, W_attn, b_attn, W_proj, b_proj)
    res = run_bass_kernel_spmd(
        nc, in_maps, core_ids=list(range(8)), trace=_trace, tmpdir=_tmpdir)
    parts = [res.results[i]["out"] for i in range(8)]
    out = np.empty((x.shape[0], T, C), np.float32)
    for b in range(x.shape[0]):
        out[b] = parts[4 * b] + parts[4 * b + 1] + parts[4 * b + 2] + parts[4 * b + 3]
        out[b] += b_proj
    if _trace:
        kernel.last_exec_time_ns = res.exec_time_ns
    return out


# revision 12
# speedup vs baseline: 1.3254x; 1.1101x over previous
"""AlibiCausalSelfAttention on 8 Trainium2 NeuronCores.

Sharding: data-parallel over batch (B=2) x head-parallel over head groups
(16 heads -> 4 groups of 4). Core c handles batch c//4, heads [4*(c%4), 4*(c%4)+4).
Each core computes a partial projection output [T, C] (W_proj row-sharded);
the host sums the 4 partials per batch and adds b_proj.

Per-core kernel layout (T=2048, C=1024, D=64, 4 local heads):
  phase 1: qkT [512, T] = (Wqk^T x^T) via matmul(lhsT=Wqk chunk, rhs=xT chunk),
           v [T, 256] via matmul(lhsT=xT chunk, rhs=Wv chunk). Biases are added
           with K=1 matmuls against a ones row. q columns pre-scaled by 1/sqrt(D)
           on host. q/k stored per head as [128, T] tiles with 2 extra contraction
           rows encoding ALiBi: St = k_aug^T q_aug = q.k/8 + slope*(j-i) exactly
           (slopes are powers of two).
  phase 2: per (head, i-tile of 512): St[j,i] chunks [128, W], exp on ACT,
           causal diag squares masked by multiplying an upper-triangular 0/1
           matrix, PV accumulated as yT[d, i] with an appended ones column in v
           producing the softmax denominator in psum row 64. Normalization:
           reciprocal of the denom row, broadcast across partitions with a K=1
           matmul, multiplied into yT.
  phase 3: out[t, e] = yT^T Wp via matmul(lhsT=yT chunk, rhs=Wp chunk).

Matmul operands are bitcast to float32r (full-rate fp32 matmul mode).
"""

import sys

if "/opt/trn_rl_repo" not in sys.path:
    sys.path.insert(0, "/opt/trn_rl_repo")

import numpy as np

T = 2048
C = 1024
H = 16
D = 64
HL = 4          # heads per core
HD = HL * D     # 256 local head dims
IW = 512        # i-tile width
NEG = None      # causal handled structurally, no big-negative constant needed

_CACHE = {}


def _build_nc(mm_dt_name="float32r"):
    import concourse.mybir as mybir
    import concourse.tile as tile
    from concourse import bacc
    from contextlib import ExitStack

    f32 = mybir.dt.float32
    fr = mybir.dt.float16
    mm_dt = fr
    Exp = mybir.ActivationFunctionType.Exp

    nc = bacc.Bacc("TRN2", target_bir_lowering=False, debug=False, num_devices=8)

    xT = nc.dram_tensor("xT", [C, T], fr, kind="ExternalInput").ap()
    wqk = nc.dram_tensor("wqk", [C, 2 * HD], fr, kind="ExternalInput").ap()
    bqk = nc.dram_tensor("bqk", [1, 2 * HD], fr, kind="ExternalInput").ap()
    wv = nc.dram_tensor("wv", [C, HD], fr, kind="ExternalInput").ap()
    bv = nc.dram_tensor("bv", [1, HD], fr, kind="ExternalInput").ap()
    wp = nc.dram_tensor("wp", [HD, C], fr, kind="ExternalInput").ap()
    aq = nc.dram_tensor("aq", [2 * HL, T], fr, kind="ExternalInput").ap()
    ak = nc.dram_tensor("ak", [2, T], fr, kind="ExternalInput").ap()
    upx_d = nc.dram_tensor("upx", [128, IW], f32, kind="ExternalInput").ap()
    onec = nc.dram_tensor("onec", [128, 1], fr, kind="ExternalInput").ap()
    onesr_d = nc.dram_tensor("onesr", [1, IW], fr, kind="ExternalInput").ap()
    out = nc.dram_tensor("out", [T, C], f32, kind="ExternalOutput").ap()

    def r(ap):
        return ap

    with tile.TileContext(nc) as tc, ExitStack() as ctx:
        pers = ctx.enter_context(tc.tile_pool(name="pers", bufs=1))

        qaug = [pers.tile([128, T], fr, tag=f"qaug{h}", name=f"qaug{h}") for h in range(HL)]
        kaug = [pers.tile([128, T], fr, tag=f"kaug{h}", name=f"kaug{h}") for h in range(HL)]
        # v tiles per (head, t-chunk): [128, 65], col 64 = ones (denominator trick)
        vaug = [[pers.tile([128, 65], fr, tag=f"vaug{h}_{t}", name=f"vaug{h}_{t}") for t in range(T // 128)]
                for h in range(HL)]
        yT = [pers.tile([128, T], fr, tag=f"yT{i}", name=f"yT{i}") for i in range(HL // 2)]
        wp_sb = [pers.tile([128, C], fr, tag=f"wp{i}", name=f"wp{i}") for i in range(2)]
        upx = pers.tile([128, IW], f32, tag="upx")
        onescol = pers.tile([128, 1], fr, tag="onescol")
        onesrow = pers.tile([1, IW], fr, tag="onesrow")
        nc.sync.dma_start(upx[:], upx_d[:])
        nc.sync.dma_start(onescol[:], onec[:])
        nc.sync.dma_start(onesrow[:], onesr_d[:])
        for i in range(2):
            nc.sync.dma_start(wp_sb[i][:], wp[i * 128:(i + 1) * 128, :])
        for h in range(HL):
            # zero the padding rows once; alibi rows come from host arrays
            nc.sync.dma_start(qaug[h][64:66, :], aq[2 * h:2 * h + 2, :])
            nc.sync.dma_start(kaug[h][64:66, :], ak[:, :])
            for t in range(T // 128):
                nc.vector.tensor_copy(vaug[h][t][:, 64:65], onescol[:])

        # ---------------- phase 1: projections ----------------
        with tc.tile_pool(name="ph1", bufs=1) as ph1, \
             tc.tile_pool(name="ps1", bufs=4, space="PSUM") as ps1:
            xs = []
            for k in range(8):
                t_ = ph1.tile([128, T], fr, tag=f"x{k}", name=f"x{k}")
                nc.sync.dma_start(t_[:], xT[k * 128:(k + 1) * 128, :])
                xs.append(t_)
            wqks = []
            for k in range(8):
                t_ = ph1.tile([128, 2 * HD], fr, tag=f"wqk{k}", name=f"wqk{k}")
                nc.sync.dma_start(t_[:], wqk[k * 128:(k + 1) * 128, :])
                wqks.append(t_)
            wvs = []
            for k in range(8):
                t_ = ph1.tile([128, HD], fr, tag=f"wv{k}", name=f"wv{k}")
                nc.sync.dma_start(t_[:], wv[k * 128:(k + 1) * 128, :])
                wvs.append(t_)
            bqk_sb = ph1.tile([1, 2 * HD], fr, tag="bqk")
            nc.sync.dma_start(bqk_sb[:], bqk[:])
            bv_sb = ph1.tile([1, HD], fr, tag="bv")
            nc.sync.dma_start(bv_sb[:], bv[:])

            # qkT [512, T]: col-chunk cc (0,1 = q heads; 2,3 = k heads)
            for cc in range(4):
                for tt in range(4):
                    ps = ps1.tile([128, IW], f32, tag="qk", name="qkps")
                    for k in range(8):
                        nc.tensor.matmul(
                            ps[:],
                            r(wqks[k][:, cc * 128:(cc + 1) * 128]),
                            r(xs[k][:, tt * IW:(tt + 1) * IW]),
                            start=(k == 0), stop=False)
                    nc.tensor.matmul(
                        ps[:], r(bqk_sb[:, cc * 128:(cc + 1) * 128]), r(onesrow[:, 0:IW]),
                        start=False, stop=True)
                    for half in range(2):
                        h = (cc % 2) * 2 + half
                        dst = qaug[h] if cc < 2 else kaug[h]
                        nc.vector.tensor_copy(
                            dst[0:64, tt * IW:(tt + 1) * IW],
                            ps[half * 64:(half + 1) * 64, :])

            # v [T, 256] natural layout
            for t16 in range(T // 128):
                ps = ps1.tile([128, HD], f32, tag="v", name="vps")
                for k in range(8):
                    nc.tensor.matmul(
                        ps[:],
                        r(xs[k][:, t16 * 128:(t16 + 1) * 128]),
                        r(wvs[k][:]),
                        start=(k == 0), stop=False)
                nc.tensor.matmul(
                    ps[:], r(onesrow[:, 0:128]), r(bv_sb[:]), start=False, stop=True)
                for h in range(HL):
                    nc.vector.tensor_copy(
                        vaug[h][t16][:, 0:64], ps[:, h * 64:(h + 1) * 64])

        # ---------------- phase 2: attention ----------------
        with tc.tile_pool(name="ph2", bufs=3) as ph2, \
             tc.tile_pool(name="ps2", bufs=2, space="PSUM") as ps2, \
             tc.tile_pool(name="py", bufs=2, space="PSUM") as py:
            for h in range(HL):
                for it in range(T // IW):
                    i0 = it * IW
                    njc = i0 // 128 + IW // 128
                    yacc = py.tile([65, IW], f32, tag="yacc", name="yacc")
                    npair = njc // 2
                    for pj in range(npair):
                        j0a = (2 * pj) * 128
                        j0b = j0a + 128
                        c0a = max(0, j0a - i0)
                        c0b = max(0, j0b - i0)
                        Wa = IW - c0a
                        Wb = IW - c0b
                        st2 = ps2.tile([128, 2 * IW], f32, tag="st", name="st")
                        nc.tensor.matmul(
                            st2[:, 0:Wa],
                            r(kaug[h][0:66, j0a:j0a + 128]),
                            r(qaug[h][0:66, i0 + c0a:i0 + IW]),
                            start=True, stop=True)
                        nc.tensor.matmul(
                            st2[:, IW:IW + Wb],
                            r(kaug[h][0:66, j0b:j0b + 128]),
                            r(qaug[h][0:66, i0 + c0b:i0 + IW]),
                            start=True, stop=True)
                        p = ph2.tile([128, 2 * IW], fr, tag="p", name="p")
                        if j0a >= i0:
                            p32 = ph2.tile([128, 2 * IW], f32, tag="p32", name="p32")
                            nc.scalar.activation(p32[:], st2[:], Exp)
                            nc.vector.tensor_mul(p[:, 0:Wa], p32[:, 0:Wa], upx[:, 0:Wa])
                            nc.vector.tensor_mul(p[:, IW:IW + Wb], p32[:, IW:IW + Wb],
                                                 upx[:, 0:Wb])
                        else:
                            nc.scalar.activation(p[:], st2[:], Exp)
                        nc.tensor.matmul(
                            yacc[:, c0a:IW], r(vaug[h][2 * pj][:]), r(p[:, 0:Wa]),
                            start=(pj == 0), stop=False)
                        nc.tensor.matmul(
                            yacc[:, c0b:IW], r(vaug[h][2 * pj + 1][:]), r(p[:, IW:IW + Wb]),
                            start=False, stop=(pj == npair - 1))
                    dcp = ph2.tile([1, IW], f32, tag="dcp", name="dcp")
                    nc.vector.tensor_copy(dcp[:], yacc[64:65, :])
                    rec = ph2.tile([1, IW], f32, tag="rec", name="rec")
                    scr = ph2.tile([1, IW], f32, tag="scr", name="scr")
                    nc.vector.reciprocal_approx_accurate(rec[:], dcp[:], scr[:])
                    recr = ph2.tile([1, IW], fr, tag="recr", name="recr")
                    nc.vector.tensor_copy(recr[:], rec[:])
                    r64 = ps2.tile([64, IW], f32, tag="r64", name="r64")
                    nc.tensor.matmul(r64[:], r(onesrow[:, 0:64]), r(recr[:]),
                                     start=True, stop=True)
                    r64s = ph2.tile([64, IW], f32, tag="r64s", name="r64s")
                    nc.vector.tensor_copy(r64s[:], r64[:])
                    nc.vector.tensor_mul(
                        yT[h // 2][(h % 2) * 64:(h % 2) * 64 + 64, i0:i0 + IW],
                        yacc[0:64, :], r64s[:])

        # ---------------- phase 3: output projection ----------------
        with tc.tile_pool(name="ph3", bufs=3) as ph3, \
             tc.tile_pool(name="ps3", bufs=2, space="PSUM") as ps3:
            for t16 in range(T // 128):
                for e2 in range(2):
                    ps = ps3.tile([128, 512], f32, tag="o", name="ops")
                    for kk in range(2):
                        nc.tensor.matmul(
                            ps[:],
                            r(yT[kk][:, t16 * 128:(t16 + 1) * 128]),
                            r(wp_sb[kk][:, e2 * 512:(e2 + 1) * 512]),
                            start=(kk == 0), stop=(kk == 1))
                    ot = ph3.tile([128, 512], f32, tag="ot", name="ot")
                    nc.scalar.copy(ot[:], ps[:])
                    nc.sync.dma_start(
                        out[t16 * 128:(t16 + 1) * 128, e2 * 512:(e2 + 1) * 512],
                        ot[:])

    nc.compile()
    return nc


def _get_nc():
    if "nc" not in _CACHE:
        _CACHE["nc"] = _build_nc()
    return _CACHE["nc"]


def _shard_inputs(x, W_attn, b_attn, W_proj, b_proj):
    f16 = np.float16
    slopes = (1.0 / np.power(2.0, np.arange(1, H + 1))).astype(np.float32)
    iota = np.arange(T, dtype=np.float32)
    ak = np.stack([np.ones(T, np.float32), iota]).astype(f16)      # [2, T]
    # multiplicative mask for diagonal chunks: cols 0:128 upper-tri keep, rest ones
    pp, ff = np.meshgrid(np.arange(128), np.arange(128), indexing="ij")
    upx = np.ones((128, IW), np.float32)
    upx[:, 0:128] = np.where(pp <= ff, 1.0, 0.0)
    xTs = [np.ascontiguousarray(x[b].T).astype(f16) for b in range(x.shape[0])]

    in_maps = []
    for core in range(8):
        b, g = core // 4, core % 4
        cs = slice(g * HD, (g + 1) * HD)
        q_cols = W_attn[:, 0:C][:, cs] * 0.125
        k_cols = W_attn[:, C:2 * C][:, cs]
        v_cols = np.ascontiguousarray(W_attn[:, 2 * C:3 * C][:, cs])
        wqk_l = np.ascontiguousarray(np.concatenate([q_cols, k_cols], axis=1))
        bqk_l = np.concatenate(
            [b_attn[0:C][cs] * 0.125, b_attn[C:2 * C][cs]])[None, :]
        bv_l = b_attn[2 * C:3 * C][cs][None, :]
        wp_l = np.ascontiguousarray(W_proj[g * HD:(g + 1) * HD, :])
        aq = np.zeros((2 * HL, T), np.float32)
        for hh in range(HL):
            s = slopes[g * HL + hh]
            aq[2 * hh, :] = -s * iota
            aq[2 * hh + 1, :] = s
        in_maps.append({
            "xT": xTs[b], "wqk": wqk_l.astype(f16),
            "bqk": np.ascontiguousarray(bqk_l).astype(f16),
            "wv": v_cols.astype(f16), "bv": np.ascontiguousarray(bv_l).astype(f16),
            "wp": wp_l.astype(f16), "aq": aq.astype(f16), "ak": ak,
            "upx": upx, "onec": np.ones((128, 1), f16),
            "onesr": np.ones((1, IW), f16),
        })
    return in_maps


def kernel(x, W_attn, b_attn, W_proj, b_proj, _trace=False, _tmpdir=None):
    from concourse.bass_utils import run_bass_kernel_spmd

    x = np.asarray(x, dtype=np.float32)
    W_attn = np.asarray(W_attn, dtype=np.float32)
    b_attn = np.asarray(b_attn, dtype=np.float32)
    W_proj = np.asarray(W_proj, dtype=np.float32)
    b_proj = np.asarray(b_proj, dtype=np.float32)

    nc = _get_nc()
    in_maps = _shard_inputs(x, W_attn, b_attn, W_proj, b_proj)
    res = run_bass_kernel_spmd(
        nc, in_maps, core_ids=list(range(8)), trace=_trace, tmpdir=_tmpdir)
    parts = [res.results[i]["out"] for i in range(8)]
    out = np.empty((x.shape[0], T, C), np.float32)
    for b in range(x.shape[0]):
        out[b] = parts[4 * b] + parts[4 * b + 1] + parts[4 * b + 2] + parts[4 * b + 3]
        out[b] += b_proj
    if _trace:
        kernel.last_exec_time_ns = res.exec_time_ns
    return out


# revision 13
# speedup vs baseline: 1.3312x; 1.0043x over previous
"""AlibiCausalSelfAttention on 8 Trainium2 NeuronCores.

Sharding: data-parallel over batch (B=2) x head-parallel over head groups
(16 heads -> 4 groups of 4). Core c handles batch c//4, heads [4*(c%4), 4*(c%4)+4).
Each core computes a partial projection output [T, C] (W_proj row-sharded);
the host sums the 4 partials per batch and adds b_proj.

Per-core kernel layout (T=2048, C=1024, D=64, 4 local heads):
  phase 1: qkT [512, T] = (Wqk^T x^T) via matmul(lhsT=Wqk chunk, rhs=xT chunk),
           v [T, 256] via matmul(lhsT=xT chunk, rhs=Wv chunk). Biases are added
           with K=1 matmuls against a ones row. q columns pre-scaled by 1/sqrt(D)
           on host. q/k stored per head as [128, T] tiles with 2 extra contraction
           rows encoding ALiBi: St = k_aug^T q_aug = q.k/8 + slope*(j-i) exactly
           (slopes are powers of two).
  phase 2: per (head, i-tile of 512): St[j,i] chunks [128, W], exp on ACT,
           causal diag squares masked by multiplying an upper-triangular 0/1
           matrix, PV accumulated as yT[d, i] with an appended ones column in v
           producing the softmax denominator in psum row 64. Normalization:
           reciprocal of the denom row, broadcast across partitions with a K=1
           matmul, multiplied into yT.
  phase 3: out[t, e] = yT^T Wp via matmul(lhsT=yT chunk, rhs=Wp chunk).

Matmul operands are bitcast to float32r (full-rate fp32 matmul mode).
"""

import sys

if "/opt/trn_rl_repo" not in sys.path:
    sys.path.insert(0, "/opt/trn_rl_repo")

import numpy as np

T = 2048
C = 1024
H = 16
D = 64
HL = 4          # heads per core
HD = HL * D     # 256 local head dims
IW = 512        # i-tile width
NEG = None      # causal handled structurally, no big-negative constant needed

_CACHE = {}


def _build_nc(mm_dt_name="float32r"):
    import concourse.mybir as mybir
    import concourse.tile as tile
    from concourse import bacc
    from contextlib import ExitStack

    f32 = mybir.dt.float32
    fr = mybir.dt.float16
    mm_dt = fr
    Exp = mybir.ActivationFunctionType.Exp

    nc = bacc.Bacc("TRN2", target_bir_lowering=False, debug=False, num_devices=8)

    xT = nc.dram_tensor("xT", [C, T], fr, kind="ExternalInput").ap()
    wqk = nc.dram_tensor("wqk", [C, 2 * HD], fr, kind="ExternalInput").ap()
    bqk = nc.dram_tensor("bqk", [1, 2 * HD], fr, kind="ExternalInput").ap()
    wv = nc.dram_tensor("wv", [C, HD], fr, kind="ExternalInput").ap()
    bv = nc.dram_tensor("bv", [1, HD], fr, kind="ExternalInput").ap()
    wp = nc.dram_tensor("wp", [HD, C], fr, kind="ExternalInput").ap()
    aq = nc.dram_tensor("aq", [2 * HL, T], fr, kind="ExternalInput").ap()
    ak = nc.dram_tensor("ak", [2, T], fr, kind="ExternalInput").ap()
    upx_d = nc.dram_tensor("upx", [128, IW], f32, kind="ExternalInput").ap()
    onec = nc.dram_tensor("onec", [128, 1], fr, kind="ExternalInput").ap()
    onesr_d = nc.dram_tensor("onesr", [1, IW], fr, kind="ExternalInput").ap()
    out = nc.dram_tensor("out", [T, C], f32, kind="ExternalOutput").ap()

    def r(ap):
        return ap

    with tile.TileContext(nc) as tc, ExitStack() as ctx:
        pers = ctx.enter_context(tc.tile_pool(name="pers", bufs=1))

        qaug = [pers.tile([128, T], fr, tag=f"qaug{h}", name=f"qaug{h}") for h in range(HL)]
        kaug = [pers.tile([128, T], fr, tag=f"kaug{h}", name=f"kaug{h}") for h in range(HL)]
        # v tiles per (head, t-chunk): [128, 65], col 64 = ones (denominator trick)
        vaug = [[pers.tile([128, 65], fr, tag=f"vaug{h}_{t}", name=f"vaug{h}_{t}") for t in range(T // 128)]
                for h in range(HL)]
        yT = [pers.tile([128, T], fr, tag=f"yT{i}", name=f"yT{i}") for i in range(HL // 2)]
        wp_sb = [pers.tile([128, C], fr, tag=f"wp{i}", name=f"wp{i}") for i in range(2)]
        upx = pers.tile([128, IW], f32, tag="upx")
        onescol = pers.tile([128, 1], fr, tag="onescol")
        onesrow = pers.tile([1, IW], fr, tag="onesrow")
        nc.sync.dma_start(upx[:], upx_d[:])
        nc.sync.dma_start(onescol[:], onec[:])
        nc.sync.dma_start(onesrow[:], onesr_d[:])
        for i in range(2):
            nc.sync.dma_start(wp_sb[i][:], wp[i * 128:(i + 1) * 128, :])
        for h in range(HL):
            # zero the padding rows once; alibi rows come from host arrays
            nc.sync.dma_start(qaug[h][64:66, :], aq[2 * h:2 * h + 2, :])
            nc.sync.dma_start(kaug[h][64:66, :], ak[:, :])
            for t in range(T // 128):
                nc.vector.tensor_copy(vaug[h][t][:, 64:65], onescol[:])

        # ---------------- phase 1: projections ----------------
        with tc.tile_pool(name="ph1", bufs=1) as ph1, \
             tc.tile_pool(name="ps1", bufs=4, space="PSUM") as ps1:
            xs, wqks, wvs = [], [], []
            bqk_sb = ph1.tile([1, 2 * HD], fr, tag="bqk")
            nc.sync.dma_start(bqk_sb[:], bqk[:])
            bv_sb = ph1.tile([1, HD], fr, tag="bv")
            nc.sync.dma_start(bv_sb[:], bv[:])
            for k in range(8):
                tw = ph1.tile([128, 2 * HD], fr, tag=f"wqk{k}", name=f"wqk{k}")
                nc.sync.dma_start(tw[:], wqk[k * 128:(k + 1) * 128, :])
                wqks.append(tw)
                tx = ph1.tile([128, T], fr, tag=f"x{k}", name=f"x{k}")
                nc.sync.dma_start(tx[:], xT[k * 128:(k + 1) * 128, :])
                xs.append(tx)
                tv = ph1.tile([128, HD], fr, tag=f"wv{k}", name=f"wv{k}")
                nc.sync.dma_start(tv[:], wv[k * 128:(k + 1) * 128, :])
                wvs.append(tv)

            # qkT [512, T]: col-chunk cc (0,1 = q heads; 2,3 = k heads)
            for cc in range(4):
                for tt in range(4):
                    ps = ps1.tile([128, IW], f32, tag="qk", name="qkps")
                    for k in range(8):
                        nc.tensor.matmul(
                            ps[:],
                            r(wqks[k][:, cc * 128:(cc + 1) * 128]),
                            r(xs[k][:, tt * IW:(tt + 1) * IW]),
                            start=(k == 0), stop=False)
                    nc.tensor.matmul(
                        ps[:], r(bqk_sb[:, cc * 128:(cc + 1) * 128]), r(onesrow[:, 0:IW]),
                        start=False, stop=True)
                    for half in range(2):
                        h = (cc % 2) * 2 + half
                        dst = qaug[h] if cc < 2 else kaug[h]
                        nc.vector.tensor_copy(
                            dst[0:64, tt * IW:(tt + 1) * IW],
                            ps[half * 64:(half + 1) * 64, :])

            # v [T, 256] natural layout
            for t16 in range(T // 128):
                ps = ps1.tile([128, HD], f32, tag="v", name="vps")
                for k in range(8):
                    nc.tensor.matmul(
                        ps[:],
                        r(xs[k][:, t16 * 128:(t16 + 1) * 128]),
                        r(wvs[k][:]),
                        start=(k == 0), stop=False)
                nc.tensor.matmul(
                    ps[:], r(onesrow[:, 0:128]), r(bv_sb[:]), start=False, stop=True)
                for h in range(HL):
                    nc.vector.tensor_copy(
                        vaug[h][t16][:, 0:64], ps[:, h * 64:(h + 1) * 64])

        # ---------------- phase 2: attention ----------------
        with tc.tile_pool(name="ph2", bufs=3) as ph2, \
             tc.tile_pool(name="ps2", bufs=2, space="PSUM") as ps2, \
             tc.tile_pool(name="py", bufs=2, space="PSUM") as py:
            for h in range(HL):
                for it in range(T // IW):
                    i0 = it * IW
                    njc = i0 // 128 + IW // 128
                    yacc = py.tile([65, IW], f32, tag="yacc", name="yacc")
                    npair = njc // 2
                    for pj in range(npair):
                        j0a = (2 * pj) * 128
                        j0b = j0a + 128
                        c0a = max(0, j0a - i0)
                        c0b = max(0, j0b - i0)
                        Wa = IW - c0a
                        Wb = IW - c0b
                        st2 = ps2.tile([128, 2 * IW], f32, tag="st", name="st")
                        nc.tensor.matmul(
                            st2[:, 0:Wa],
                            r(kaug[h][0:66, j0a:j0a + 128]),
                            r(qaug[h][0:66, i0 + c0a:i0 + IW]),
                            start=True, stop=True)
                        nc.tensor.matmul(
                            st2[:, IW:IW + Wb],
                            r(kaug[h][0:66, j0b:j0b + 128]),
                            r(qaug[h][0:66, i0 + c0b:i0 + IW]),
                            start=True, stop=True)
                        p = ph2.tile([128, 2 * IW], fr, tag="p", name="p")
                        if j0a >= i0:
                            p32 = ph2.tile([128, 2 * IW], f32, tag="p32", name="p32")
                            nc.scalar.activation(p32[:], st2[:], Exp)
                            nc.vector.tensor_mul(p[:, 0:Wa], p32[:, 0:Wa], upx[:, 0:Wa])
                            nc.vector.tensor_mul(p[:, IW:IW + Wb], p32[:, IW:IW + Wb],
                                                 upx[:, 0:Wb])
                        else:
                            nc.scalar.activation(p[:], st2[:], Exp)
                        nc.tensor.matmul(
                            yacc[:, c0a:IW], r(vaug[h][2 * pj][:]), r(p[:, 0:Wa]),
                            start=(pj == 0), stop=False)
                        nc.tensor.matmul(
                            yacc[:, c0b:IW], r(vaug[h][2 * pj + 1][:]), r(p[:, IW:IW + Wb]),
                            start=False, stop=(pj == npair - 1))
                    dcp = ph2.tile([1, IW], f32, tag="dcp", name="dcp")
                    nc.vector.tensor_copy(dcp[:], yacc[64:65, :])
                    rec = ph2.tile([1, IW], f32, tag="rec", name="rec")
                    scr = ph2.tile([1, IW], f32, tag="scr", name="scr")
                    nc.vector.reciprocal_approx_accurate(rec[:], dcp[:], scr[:])
                    recr = ph2.tile([1, IW], fr, tag="recr", name="recr")
                    nc.vector.tensor_copy(recr[:], rec[:])
                    r64 = ps2.tile([64, IW], f32, tag="r64", name="r64")
                    nc.tensor.matmul(r64[:], r(onesrow[:, 0:64]), r(recr[:]),
                                     start=True, stop=True)
                    r64s = ph2.tile([64, IW], f32, tag="r64s", name="r64s")
                    nc.vector.tensor_copy(r64s[:], r64[:])
                    nc.vector.tensor_mul(
                        yT[h // 2][(h % 2) * 64:(h % 2) * 64 + 64, i0:i0 + IW],
                        yacc[0:64, :], r64s[:])

        # ---------------- phase 3: output projection ----------------
        with tc.tile_pool(name="ph3", bufs=3) as ph3, \
             tc.tile_pool(name="ps3", bufs=2, space="PSUM") as ps3:
            for t16 in range(T // 128):
                for e2 in range(2):
                    ps = ps3.tile([128, 512], f32, tag="o", name="ops")
                    for kk in range(2):
                        nc.tensor.matmul(
                            ps[:],
                            r(yT[kk][:, t16 * 128:(t16 + 1) * 128]),
                            r(wp_sb[kk][:, e2 * 512:(e2 + 1) * 512]),
                            start=(kk == 0), stop=(kk == 1))
                    ot = ph3.tile([128, 512], f32, tag="ot", name="ot")
                    nc.scalar.copy(ot[:], ps[:])
                    nc.sync.dma_start(
                        out[t16 * 128:(t16 + 1) * 128, e2 * 512:(e2 + 1) * 512],
                        ot[:])

    nc.compile()
    return nc


def _get_nc():
    if "nc" not in _CACHE:
        _CACHE["nc"] = _build_nc()
    return _CACHE["nc"]


def _shard_inputs(x, W_attn, b_attn, W_proj, b_proj):
    f16 = np.float16
    slopes = (1.0 / np.power(2.0, np.arange(1, H + 1))).astype(np.float32)
    iota = np.arange(T, dtype=np.float32)
    ak = np.stack([np.ones(T, np.float32), iota]).astype(f16)      # [2, T]
    # multiplicative mask for diagonal chunks: cols 0:128 upper-tri keep, rest ones
    pp, ff = np.meshgrid(np.arange(128), np.arange(128), indexing="ij")
    upx = np.ones((128, IW), np.float32)
    upx[:, 0:128] = np.where(pp <= ff, 1.0, 0.0)
    xTs = [np.ascontiguousarray(x[b].T).astype(f16) for b in range(x.shape[0])]

    in_maps = []
    for core in range(8):
        b, g = core // 4, core % 4
        cs = slice(g * HD, (g + 1) * HD)
        q_cols = W_attn[:, 0:C][:, cs] * 0.125
        k_cols = W_attn[:, C:2 * C][:, cs]
        v_cols = np.ascontiguousarray(W_attn[:, 2 * C:3 * C][:, cs])
        wqk_l = np.ascontiguousarray(np.concatenate([q_cols, k_cols], axis=1))
        bqk_l = np.concatenate(
            [b_attn[0:C][cs] * 0.125, b_attn[C:2 * C][cs]])[None, :]
        bv_l = b_attn[2 * C:3 * C][cs][None, :]
        wp_l = np.ascontiguousarray(W_proj[g * HD:(g + 1) * HD, :])
        aq = np.zeros((2 * HL, T), np.float32)
        for hh in range(HL):
            s = slopes[g * HL + hh]
            aq[2 * hh, :] = -s * iota
            aq[2 * hh + 1, :] = s
        in_maps.append({
            "xT": xTs[b], "wqk": wqk_l.astype(f16),
            "bqk": np.ascontiguousarray(bqk_l).astype(f16),
            "wv": v_cols.astype(f16), "bv": np.ascontiguousarray(bv_l).astype(f16),
            "wp": wp_l.astype(f16), "aq": aq.astype(f16), "ak": ak,
            "upx": upx, "onec": np.ones((128, 1), f16),
            "onesr": np.ones((1, IW), f16),
        })
    return in_maps


def kernel(x, W_attn, b_attn, W_proj, b_proj, _trace=False, _tmpdir=None):
    from concourse.bass_utils import run_bass_kernel_spmd

    x = np.asarray(x, dtype=np.float32)
    W_attn = np.asarray(W_attn, dtype=np.float32)
    b_attn = np.asarray(b_attn, dtype=np.float32)
    W_proj = np.asarray(W_proj, dtype=np.float32)
    b_proj = np.asarray(b_proj, dtype=np.float32)

    nc = _get_nc()
    in_maps = _shard_inputs(x, W_attn, b_attn, W_proj, b_proj)
    res = run_bass_kernel_spmd(
        nc, in_maps, core_ids=list(range(8)), trace=_trace, tmpdir=_tmpdir)
    parts = [res.results[i]["out"] for i in range(8)]
    out = np.empty((x.shape[0], T, C), np.float32)
    for b in range(x.shape[0]):
        out[b] = parts[4 * b] + parts[4 * b + 1] + parts[4 * b + 2] + parts[4 * b + 3]
        out[b] += b_proj
    if _trace:
        kernel.last_exec_time_ns = res.exec_time_ns
    return out


# revision 14
# speedup vs baseline: 1.4332x; 1.0767x over previous
"""AlibiCausalSelfAttention on 8 Trainium2 NeuronCores.

Sharding: data-parallel over batch (B=2) x head-parallel over head groups
(16 heads -> 4 groups of 4). Core c handles batch c//4, heads [4*(c%4), 4*(c%4)+4).
Each core computes a partial projection output [T, C] (W_proj row-sharded);
the host sums the 4 partials per batch and adds b_proj.

Per-core kernel layout (T=2048, C=1024, D=64, 4 local heads):
  phase 1: qkT [512, T] = (Wqk^T x^T) via matmul(lhsT=Wqk chunk, rhs=xT chunk),
           v [T, 256] via matmul(lhsT=xT chunk, rhs=Wv chunk). Biases are added
           with K=1 matmuls against a ones row. q columns pre-scaled by 1/sqrt(D)
           on host. q/k stored per head as [128, T] tiles with 2 extra contraction
           rows encoding ALiBi: St = k_aug^T q_aug = q.k/8 + slope*(j-i) exactly
           (slopes are powers of two).
  phase 2: per (head, i-tile of 512): St[j,i] chunks [128, W], exp on ACT,
           causal diag squares masked by multiplying an upper-triangular 0/1
           matrix, PV accumulated as yT[d, i] with an appended ones column in v
           producing the softmax denominator in psum row 64. Normalization:
           reciprocal of the denom row, broadcast across partitions with a K=1
           matmul, multiplied into yT.
  phase 3: out[t, e] = yT^T Wp via matmul(lhsT=yT chunk, rhs=Wp chunk).

Matmul operands are bitcast to float32r (full-rate fp32 matmul mode).
"""

import sys

if "/opt/trn_rl_repo" not in sys.path:
    sys.path.insert(0, "/opt/trn_rl_repo")

import numpy as np

T = 2048
C = 1024
H = 16
D = 64
HL = 4          # heads per core
HD = HL * D     # 256 local head dims
IW = 512        # i-tile width
NEG = None      # causal handled structurally, no big-negative constant needed

_CACHE = {}


def _build_nc(mm_dt_name="float32r"):
    import concourse.mybir as mybir
    import concourse.tile as tile
    from concourse import bacc
    from contextlib import ExitStack

    f32 = mybir.dt.float32
    fr = mybir.dt.float16
    mm_dt = fr
    Exp = mybir.ActivationFunctionType.Exp

    nc = bacc.Bacc("TRN2", target_bir_lowering=False, debug=False, num_devices=8)

    xT = nc.dram_tensor("xT", [C, T], fr, kind="ExternalInput").ap()
    wqk = nc.dram_tensor("wqk", [C, 2 * HD], fr, kind="ExternalInput").ap()
    bqk = nc.dram_tensor("bqk", [1, 2 * HD], fr, kind="ExternalInput").ap()
    wv = nc.dram_tensor("wv", [C, HD], fr, kind="ExternalInput").ap()
    bv = nc.dram_tensor("bv", [1, HD], fr, kind="ExternalInput").ap()
    wp = nc.dram_tensor("wp", [HD, C], fr, kind="ExternalInput").ap()
    aq = nc.dram_tensor("aq", [2 * HL, T], fr, kind="ExternalInput").ap()
    ak = nc.dram_tensor("ak", [2, T], fr, kind="ExternalInput").ap()
    upx_d = nc.dram_tensor("upx", [128, IW], f32, kind="ExternalInput").ap()
    onec = nc.dram_tensor("onec", [128, 1], fr, kind="ExternalInput").ap()
    onesr_d = nc.dram_tensor("onesr", [1, IW], fr, kind="ExternalInput").ap()
    out = nc.dram_tensor("out", [T, C], f32, kind="ExternalOutput").ap()

    def r(ap):
        return ap

    with tile.TileContext(nc) as tc, ExitStack() as ctx:
        pers = ctx.enter_context(tc.tile_pool(name="pers", bufs=1))

        qaug = [pers.tile([128, T], fr, tag=f"qaug{h}", name=f"qaug{h}") for h in range(HL)]
        kaug = [pers.tile([128, T], fr, tag=f"kaug{h}", name=f"kaug{h}") for h in range(HL)]
        # v tiles per (head, t-chunk): [128, 65], col 64 = ones (denominator trick)
        vaug = [[pers.tile([128, 65], fr, tag=f"vaug{h}_{t}", name=f"vaug{h}_{t}") for t in range(T // 128)]
                for h in range(HL)]
        yT = [pers.tile([128, T], fr, tag=f"yT{i}", name=f"yT{i}") for i in range(HL // 2)]
        wp_sb = [pers.tile([128, C], fr, tag=f"wp{i}", name=f"wp{i}") for i in range(2)]
        upx = pers.tile([128, IW], f32, tag="upx")
        onescol = pers.tile([128, 1], fr, tag="onescol")
        onesrow = pers.tile([1, IW], fr, tag="onesrow")
        nc.sync.dma_start(upx[:], upx_d[:])
        nc.sync.dma_start(onescol[:], onec[:])
        nc.sync.dma_start(onesrow[:], onesr_d[:])
        for i in range(2):
            nc.sync.dma_start(wp_sb[i][:], wp[i * 128:(i + 1) * 128, :])
        for h in range(HL):
            # zero the padding rows once; alibi rows come from host arrays
            nc.sync.dma_start(qaug[h][64:66, :], aq[2 * h:2 * h + 2, :])
            nc.sync.dma_start(kaug[h][64:66, :], ak[:, :])
            for t in range(T // 128):
                nc.vector.tensor_copy(vaug[h][t][:, 64:65], onescol[:])

        # ---------------- phase 1: projections ----------------
        with tc.tile_pool(name="ph1", bufs=1) as ph1, \
             tc.tile_pool(name="ps1", bufs=4, space="PSUM") as ps1:
            xs, wqks, wvs = [], [], []
            bqk_sb = ph1.tile([1, 2 * HD], fr, tag="bqk")
            nc.sync.dma_start(bqk_sb[:], bqk[:])
            bv_sb = ph1.tile([1, HD], fr, tag="bv")
            nc.sync.dma_start(bv_sb[:], bv[:])
            for k in range(8):
                tw = ph1.tile([128, 2 * HD], fr, tag=f"wqk{k}", name=f"wqk{k}")
                nc.sync.dma_start(tw[:], wqk[k * 128:(k + 1) * 128, :])
                wqks.append(tw)
                tx = ph1.tile([128, T], fr, tag=f"x{k}", name=f"x{k}")
                nc.sync.dma_start(tx[:], xT[k * 128:(k + 1) * 128, :])
                xs.append(tx)
                tv = ph1.tile([128, HD], fr, tag=f"wv{k}", name=f"wv{k}")
                nc.sync.dma_start(tv[:], wv[k * 128:(k + 1) * 128, :])
                wvs.append(tv)

            # qkT [512, T]: col-chunk cc (0,1 = q heads; 2,3 = k heads)
            for cc in range(4):
                for tt in range(4):
                    ps = ps1.tile([128, IW], f32, tag="qk", name="qkps")
                    for k in range(8):
                        nc.tensor.matmul(
                            ps[:],
                            r(wqks[k][:, cc * 128:(cc + 1) * 128]),
                            r(xs[k][:, tt * IW:(tt + 1) * IW]),
                            start=(k == 0), stop=False)
                    nc.tensor.matmul(
                        ps[:], r(bqk_sb[:, cc * 128:(cc + 1) * 128]), r(onesrow[:, 0:IW]),
                        start=False, stop=True)
                    for half in range(2):
                        h = (cc % 2) * 2 + half
                        dst = qaug[h] if cc < 2 else kaug[h]
                        nc.vector.tensor_copy(
                            dst[0:64, tt * IW:(tt + 1) * IW],
                            ps[half * 64:(half + 1) * 64, :])

            # v [T, 256] natural layout
            for t16 in range(T // 128):
                ps = ps1.tile([128, HD], f32, tag="v", name="vps")
                for k in range(8):
                    nc.tensor.matmul(
                        ps[:],
                        r(xs[k][:, t16 * 128:(t16 + 1) * 128]),
                        r(wvs[k][:]),
                        start=(k == 0), stop=False)
                nc.tensor.matmul(
                    ps[:], r(onesrow[:, 0:128]), r(bv_sb[:]), start=False, stop=True)
                for h in range(HL):
                    nc.vector.tensor_copy(
                        vaug[h][t16][:, 0:64], ps[:, h * 64:(h + 1) * 64])

        # ------- phase 2+3: attention with interleaved output projection -------
        with tc.tile_pool(name="ph2", bufs=3) as ph2, \
             tc.tile_pool(name="ps2", bufs=2, space="PSUM") as ps2, \
             tc.tile_pool(name="py", bufs=2, space="PSUM") as py, \
             tc.tile_pool(name="pr", bufs=1, space="PSUM") as pr, \
             tc.tile_pool(name="po", bufs=1, space="PSUM") as po, \
             tc.tile_pool(name="ph3", bufs=3) as ph3:
            for it in range(T // IW):
                i0 = it * IW
                njc = i0 // 128 + IW // 128
                for h in range(HL):
                    yacc = py.tile([65, IW], f32, tag="yacc", name="yacc")
                    npair = njc // 2
                    for pj in range(npair):
                        j0a = (2 * pj) * 128
                        j0b = j0a + 128
                        c0a = max(0, j0a - i0)
                        c0b = max(0, j0b - i0)
                        Wa = IW - c0a
                        Wb = IW - c0b
                        st2 = ps2.tile([128, 2 * IW], f32, tag="st", name="st")
                        nc.tensor.matmul(
                            st2[:, 0:Wa],
                            r(kaug[h][0:66, j0a:j0a + 128]),
                            r(qaug[h][0:66, i0 + c0a:i0 + IW]),
                            start=True, stop=True)
                        nc.tensor.matmul(
                            st2[:, IW:IW + Wb],
                            r(kaug[h][0:66, j0b:j0b + 128]),
                            r(qaug[h][0:66, i0 + c0b:i0 + IW]),
                            start=True, stop=True)
                        p = ph2.tile([128, 2 * IW], fr, tag="p", name="p")
                        if j0a >= i0:
                            p32 = ph2.tile([128, 2 * IW], f32, tag="p32", name="p32")
                            nc.scalar.activation(p32[:], st2[:], Exp)
                            nc.vector.tensor_mul(p[:, 0:Wa], p32[:, 0:Wa], upx[:, 0:Wa])
                            nc.vector.tensor_mul(p[:, IW:IW + Wb], p32[:, IW:IW + Wb],
                                                 upx[:, 0:Wb])
                        else:
                            nc.scalar.activation(p[:], st2[:], Exp)
                        nc.tensor.matmul(
                            yacc[:, c0a:IW], r(vaug[h][2 * pj][:]), r(p[:, 0:Wa]),
                            start=(pj == 0), stop=False)
                        nc.tensor.matmul(
                            yacc[:, c0b:IW], r(vaug[h][2 * pj + 1][:]), r(p[:, IW:IW + Wb]),
                            start=False, stop=(pj == npair - 1))
                    dcp = ph2.tile([1, IW], f32, tag="dcp", name="dcp")
                    nc.vector.tensor_copy(dcp[:], yacc[64:65, :])
                    rec = ph2.tile([1, IW], f32, tag="rec", name="rec")
                    scr = ph2.tile([1, IW], f32, tag="scr", name="scr")
                    nc.vector.reciprocal_approx_accurate(rec[:], dcp[:], scr[:])
                    recr = ph2.tile([1, IW], fr, tag="recr", name="recr")
                    nc.vector.tensor_copy(recr[:], rec[:])
                    r64 = pr.tile([64, IW], f32, tag="r64", name="r64")
                    nc.tensor.matmul(r64[:], r(onesrow[:, 0:64]), r(recr[:]),
                                     start=True, stop=True)
                    r64s = ph2.tile([64, IW], f32, tag="r64s", name="r64s")
                    nc.vector.tensor_copy(r64s[:], r64[:])
                    nc.vector.tensor_mul(
                        yT[h // 2][(h % 2) * 64:(h % 2) * 64 + 64, i0:i0 + IW],
                        yacc[0:64, :], r64s[:])
                # output projection for the t-chunks this i-tile completed
                for t16 in range(4 * it, 4 * it + 4):
                    for e2 in range(2):
                        ps = po.tile([128, 512], f32, tag="o", name="ops")
                        for kk in range(2):
                            nc.tensor.matmul(
                                ps[:],
                                r(yT[kk][:, t16 * 128:(t16 + 1) * 128]),
                                r(wp_sb[kk][:, e2 * 512:(e2 + 1) * 512]),
                                start=(kk == 0), stop=(kk == 1))
                        ot = ph3.tile([128, 512], f32, tag="ot", name="ot")
                        nc.vector.tensor_copy(ot[:], ps[:])
                        nc.sync.dma_start(
                            out[t16 * 128:(t16 + 1) * 128, e2 * 512:(e2 + 1) * 512],
                            ot[:])

    nc.compile()
    return nc


def _get_nc():
    if "nc" not in _CACHE:
        _CACHE["nc"] = _build_nc()
    return _CACHE["nc"]


def _shard_inputs(x, W_attn, b_attn, W_proj, b_proj):
    f16 = np.float16
    slopes = (1.0 / np.power(2.0, np.arange(1, H + 1))).astype(np.float32)
    iota = np.arange(T, dtype=np.float32)
    ak = np.stack([np.ones(T, np.float32), iota]).astype(f16)      # [2, T]
    # multiplicative mask for diagonal chunks: cols 0:128 upper-tri keep, rest ones
    pp, ff = np.meshgrid(np.arange(128), np.arange(128), indexing="ij")
    upx = np.ones((128, IW), np.float32)
    upx[:, 0:128] = np.where(pp <= ff, 1.0, 0.0)
    xTs = [np.ascontiguousarray(x[b].T).astype(f16) for b in range(x.shape[0])]

    in_maps = []
    for core in range(8):
        b, g = core // 4, core % 4
        cs = slice(g * HD, (g + 1) * HD)
        q_cols = W_attn[:, 0:C][:, cs] * 0.125
        k_cols = W_attn[:, C:2 * C][:, cs]
        v_cols = np.ascontiguousarray(W_attn[:, 2 * C:3 * C][:, cs])
        wqk_l = np.ascontiguousarray(np.concatenate([q_cols, k_cols], axis=1))
        bqk_l = np.concatenate(
            [b_attn[0:C][cs] * 0.125, b_attn[C:2 * C][cs]])[None, :]
        bv_l = b_attn[2 * C:3 * C][cs][None, :]
        wp_l = np.ascontiguousarray(W_proj[g * HD:(g + 1) * HD, :])
        aq = np.zeros((2 * HL, T), np.float32)
        for hh in range(HL):
            s = slopes[g * HL + hh]
            aq[2 * hh, :] = -s * iota
            aq[2 * hh + 1, :] = s
        in_maps.append({
            "xT": xTs[b], "wqk": wqk_l.astype(f16),
            "bqk": np.ascontiguousarray(bqk_l).astype(f16),
            "wv": v_cols.astype(f16), "bv": np.ascontiguousarray(bv_l).astype(f16),
            "wp": wp_l.astype(f16), "aq": aq.astype(f16), "ak": ak,
            "upx": upx, "onec": np.ones((128, 1), f16),
            "onesr": np.ones((1, IW), f16),
        })
    return in_maps


def kernel(x, W_attn, b_attn, W_proj, b_proj, _trace=False, _tmpdir=None):
    from concourse.bass_utils import run_bass_kernel_spmd

    x = np.asarray(x, dtype=np.float32)
    W_attn = np.asarray(W_attn, dtype=np.float32)
    b_attn = np.asarray(b_attn, dtype=np.float32)
    W_proj = np.asarray(W_proj, dtype=np.float32)
    b_proj = np.asarray(b_proj, dtype=np.float32)

    nc = _get_nc()
    in_maps = _shard_inputs(x, W_attn, b_attn, W_proj, b_proj)
    res = run_bass_kernel_spmd(
        nc, in_maps, core_ids=list(range(8)), trace=_trace, tmpdir=_tmpdir)
    parts = [res.results[i]["out"] for i in range(8)]
    out = np.empty((x.shape[0], T, C), np.float32)
    for b in range(x.shape[0]):
        out[b] = parts[4 * b] + parts[4 * b + 1] + parts[4 * b + 2] + parts[4 * b + 3]
        out[b] += b_proj
    if _trace:
        kernel.last_exec_time_ns = res.exec_time_ns
    return out
